# revision 26
# baseline (speedup 1.0000x reference)
"""Trainium2 Bass kernel for a binarized-weight MLP (BNN MNIST-style):

    h   = x @ sign(W1).T + b1      # fc1, binarized weights
    h   = clip(h, -1, 1)           # Hardtanh
    out = h @ W2.T + b2            # fc2

Shapes: x [8192, 784] f32, W1 [4096, 784], b1 [4096], W2 [10, 4096], b2 [10].

Strategy (data-parallel over 8 NeuronCores):
  - Shard batch 8192 -> 1024 rows/core; replicate weights.
  - All matmuls in bf16 (sign(W1) in {-1,0,+1} is exact in bf16), fp32 PSUM.
  - Bias folding: append ones-rows to x^T and put b1 (hi+lo bf16 split) as
    extra rows of the fc1 weight, so fc1 bias costs nothing. K = 784+2
    zero-padded to 896 = 7 k-tiles of 128.
  - fc1 computes h^T tiles [128 hid, 512 batch]; DVE tensor_scalar(min 1,
    max -1) applies Hardtanh and casts to bf16; fc2 accumulates
    W2^T (k-tiles [128,10]) @ h^T into a [10, 512] PSUM tile, software-
    pipelined one ht iteration behind fc1 to hide DVE latency.
  - Per-core output is out^T [10, 1024] f32; host gathers + transposes.
"""

import numpy as np
import ml_dtypes
from contextlib import ExitStack

import concourse.bass as bass
import concourse.mybir as mybir
import concourse.tile as tile
from concourse import bacc
from concourse import bass_utils

BF16_NP = ml_dtypes.bfloat16
BF16 = mybir.dt.bfloat16
F32 = mybir.dt.float32

BATCH, IN, HID, OUT = 8192, 784, 4096, 10
NCORES = 8
B_CORE = BATCH // NCORES        # 1024
NT = B_CORE // 512              # 2 batch n-tiles of 512 per core
HT = HID // 128                 # 32 hidden tiles
KT = 7                          # ceil((784+2)/128) k-tiles
K_PAD = KT * 128                # 896
N_WARMUP = 2                    # PE warm-up matmuls (HAM un-throttle)
UNPAIRED_HEAD = 2               # ht groups that run nt=0 only at the start

_CACHE = {}


def _build():
    """Build + compile the Bacc graph once per process."""
    if "nc" in _CACHE:
        return _CACHE["nc"]

    nc = bacc.Bacc("TRN2", target_bir_lowering=False, debug=False,
                   num_devices=NCORES)
    xt_d = nc.dram_tensor("xt", [NT, KT, 128, 512], BF16,
                          kind="ExternalInput").ap()
    w1_d = nc.dram_tensor("w1", [HT, 128, K_PAD], BF16,
                          kind="ExternalInput").ap()
    w2_d = nc.dram_tensor("w2", [128, HT * OUT], BF16,
                          kind="ExternalInput").ap()
    b2_d = nc.dram_tensor("b2", [OUT, 1], F32, kind="ExternalInput").ap()
    out_d = nc.dram_tensor("out", [OUT, B_CORE], F32,
                           kind="ExternalOutput").ap()

    # Raw (non-Tile) SBUF tensor for PE warm-up matmuls: contents are
    # irrelevant, so reading it uninitialized is fine and needs no producer.
    warm_sb = nc.alloc_sbuf_tensor("warm_raw", [128, 512], BF16).ap()

    with tile.TileContext(nc) as tc:
        with ExitStack() as ctx:
            wpool = ctx.enter_context(tc.tile_pool(name="w1", bufs=1))
            xpool = ctx.enter_context(tc.tile_pool(name="x", bufs=1))
            cpool = ctx.enter_context(tc.tile_pool(name="const", bufs=1))
            hpool = ctx.enter_context(tc.tile_pool(name="h", bufs=3))
            ps1pool = ctx.enter_context(
                tc.tile_pool(name="ps1", bufs=2, space="PSUM"))
            ps2pool = ctx.enter_context(
                tc.tile_pool(name="ps2", bufs=1, space="PSUM"))

            w2_sb = cpool.tile([128, HT * OUT], BF16, tag="w2")
            b2_sb = cpool.tile([OUT, 1], F32, tag="b2")
            out_sb = cpool.tile([OUT, B_CORE], F32, tag="out")

            # PE warm-up: the HAM clock gate keeps the PE at 1.2 GHz until
            # ~3.4us of sustained matmul activity. Run dummy matmuls while
            # input DMAs stream so real matmuls start at 2.4 GHz. warm_sb is
            # uninitialized (garbage bf16 is fine; results are discarded) so
            # the warm-ups have no dependencies and start right after init.
            wspool = ctx.enter_context(
                tc.tile_pool(name="pswarm", bufs=1, space="PSUM"))
            for i in range(N_WARMUP):
                pw = wspool.tile([128, 512], F32, tag="pswarm")
                nc.tensor.matmul(pw[:], warm_sb[:, 0:128], warm_sb[:],
                                 start=True, stop=True)

            # Input DMAs on two parallel HWDGE queues (sync + scalar),
            # interleaved so the first-consumed tiles land first:
            #   sync:   w1[0], x(0/1, odd kt), w1[1..31]
            #   scalar: x(0/1, even kt), w2, b2
            x_t = {}
            w1_t = []

            def w1_dma(ht):
                t = wpool.tile([128, K_PAD], BF16, tag=f"w1_{ht}",
                               name=f"w1_{ht}")
                nc.sync.dma_start(t[:], w1_d[ht])
                w1_t.append(t)

            def x_dma(nt, kt, eng):
                t = xpool.tile([128, 512], BF16, tag=f"x_{nt}_{kt}")
                eng.dma_start(t[:], xt_d[nt, kt])
                x_t[(nt, kt)] = t

            w1_dma(0)
            for kt in (1, 3):
                x_dma(0, kt, nc.sync)
            w1_dma(1)
            w1_dma(2)
            for kt in (1, 3, 5):
                x_dma(1, kt, nc.sync)
            for ht in range(3, HT):
                w1_dma(ht)
            for kt in (0, 2, 4, 6, 5):
                x_dma(0, kt, nc.scalar)
            for kt in (0, 2, 4, 6):
                x_dma(1, kt, nc.scalar)
            nc.scalar.dma_start(w2_sb[:], w2_d)
            nc.scalar.dma_start(b2_sb[:], b2_d)

            # fc1 iterates (ht, kt, nt): the two nt matmuls share the same
            # stationary lhsT back-to-back (faster weight path). The first
            # UNPAIRED_HEAD ht groups run nt=0 only — they are DMA-paced and
            # this halves the x bytes needed early; their nt=1 halves run at
            # the end when everything is resident. fc2 matmuls are pipelined
            # one group behind so the PE never waits on the DVE eviction.
            ps2 = [ps2pool.tile([OUT, 512], F32, tag=f"ps2_{nt}",
                                name=f"ps2_{nt}")
                   for nt in range(NT)]
            pending = []

            groups = ([(ht, (0,)) for ht in range(UNPAIRED_HEAD)]
                      + [(ht, (0, 1)) for ht in range(UNPAIRED_HEAD, HT)]
                      + [(ht, (1,)) for ht in range(UNPAIRED_HEAD)])
            fc2_first = {0: 0, 1: UNPAIRED_HEAD % HT}
            fc2_last = {0: HT - 1, 1: (UNPAIRED_HEAD - 1) % HT}

            def fc2_mm(pht, pnt, ph):
                nc.tensor.matmul(
                    ps2[pnt][:], w2_sb[:, pht * OUT:(pht + 1) * OUT], ph[:],
                    start=(pht == fc2_first[pnt]),
                    stop=(pht == fc2_last[pnt]),
                    skip_group_check=True)

            for ht, nts in groups:
                ps1 = {nt: ps1pool.tile([128, 512], F32, tag=f"ps1_{nt}",
                                        name=f"ps1_{ht}_{nt}")
                       for nt in nts}
                # group 0 consumes x(0,kt) in DMA-arrival order
                kt_order = (0, 1, 2, 3, 4, 6, 5) if ht == 0 and nts == (0,) \
                    else tuple(range(KT))
                for i, kt in enumerate(kt_order):
                    for nt in nts:
                        nc.tensor.matmul(
                            ps1[nt][:],
                            w1_t[ht][:, kt * 128:(kt + 1) * 128],
                            x_t[(nt, kt)][:],
                            start=(i == 0), stop=(i == KT - 1),
                            skip_group_check=True)
                for nt in nts:
                    h = hpool.tile([128, 512], BF16, tag=f"h_{nt}")
                    # Hardtanh + downcast: h = max(min(ps1, 1), -1)
                    nc.vector.tensor_scalar(
                        h[:], ps1[nt][:], 1.0, -1.0,
                        op0=mybir.AluOpType.min, op1=mybir.AluOpType.max)
                    pending.append((ht, nt, h))
                while len(pending) > NT:
                    fc2_mm(*pending.pop(0))
            for pht, pnt, ph in pending:
                fc2_mm(pht, pnt, ph)
            for nt in range(NT):
                # out = ps2 + b2 (per-partition bias), f32
                nc.scalar.activation(
                    out_sb[:, nt * 512:(nt + 1) * 512], ps2[nt][:],
                    mybir.ActivationFunctionType.Identity, bias=b2_sb[:])
            nc.sync.dma_start(out_d, out_sb[:])

    nc.compile()
    _CACHE["nc"] = nc
    return nc


def _prep_inputs(x, W1, b1, W2, b2):
    """Host-side shard + layout prep. Returns in_maps for the 8 cores."""
    x = np.asarray(x, dtype=np.float32)
    W1 = np.asarray(W1, dtype=np.float32)
    b1 = np.asarray(b1, dtype=np.float32)
    W2 = np.asarray(W2, dtype=np.float32)
    b2 = np.asarray(b2, dtype=np.float32)

    # fc1 weight, augmented with two bias rows (hi + lo bf16 split of b1),
    # zero-padded to K_PAD. Layout [ht, p, kt*128+m] = w1aug[kt*128+p, ht*128+m].
    w1aug = np.zeros((K_PAD, HID), dtype=np.float32)
    w1aug[:IN] = np.sign(W1).T
    b1_hi = b1.astype(BF16_NP).astype(np.float32)
    w1aug[IN] = b1_hi
    w1aug[IN + 1] = b1 - b1_hi
    w1_host = np.ascontiguousarray(
        w1aug.astype(BF16_NP).reshape(KT, 128, HT, 128)
        .transpose(2, 1, 0, 3).reshape(HT, 128, K_PAD))

    # fc2 weight: [p, kt*10+o] = W2[o, kt*128+p]
    w2_host = np.ascontiguousarray(
        W2.T.astype(BF16_NP).reshape(HT, 128, OUT)
        .transpose(1, 0, 2).reshape(128, HT * OUT))

    b2_host = np.ascontiguousarray(b2.reshape(OUT, 1))

    # x augmented with ones-columns matching the two b1 rows.
    x_aug = np.zeros((BATCH, K_PAD), dtype=BF16_NP)
    x_aug[:, :IN] = x.astype(BF16_NP)
    x_aug[:, IN] = 1
    x_aug[:, IN + 1] = 1

    in_maps = []
    for c in range(NCORES):
        xc = x_aug[c * B_CORE:(c + 1) * B_CORE]          # [1024, 896]
        xt = np.ascontiguousarray(
            xc.reshape(NT, 512, KT, 128).transpose(0, 2, 3, 1))
        in_maps.append({"xt": xt, "w1": w1_host, "w2": w2_host,
                        "b2": b2_host})
    return in_maps


def _gather(results):
    full = np.concatenate([np.asarray(r["out"], dtype=np.float32)
                           for r in results], axis=1)    # [10, 8192]
    return np.ascontiguousarray(full.T)                  # [8192, 10]


def run(x, W1, b1, W2, b2, trace=False, **trace_kwargs):
    import os
    nc = _build()
    in_maps = _prep_inputs(x, W1, b1, W2, b2)
    if not trace:
        # The NTFF profiling hook isn't available in every environment;
        # make sure an ambient BASS_TRACE can't pull us onto that path.
        os.environ["BASS_NEVER_TRACE"] = "1"
    else:
        os.environ.pop("BASS_NEVER_TRACE", None)
    res = bass_utils.run_bass_kernel_spmd(
        nc, in_maps, core_ids=list(range(NCORES)), trace=trace,
        **trace_kwargs)
    return _gather(res.results), res


def kernel(x, W1, b1, W2, b2):
    out, _ = run(x, W1, b1, W2, b2)
    return out
